# revision 1
# baseline (speedup 1.0000x reference)
"""Trainium2 Bass kernel for CausalGraphLayer (GCN conv + causal attention mix).

out = D^{-1/2} (A+I) D^{-1/2} x @ (W @ softmax(CA, axis=1)) + b @ softmax(CA)

Strategy (8 NeuronCores, SPMD):
 - Shard destination nodes across cores (12500 each); partition edges by dst.
 - Replicate x and the small params to every core.
 - Host builds, per core, a slot table: dst nodes degree-sorted into blocks of
   128 (PSUM partitions); slot j of block b holds the j-th in-edge's source
   index and norm for each of the 128 dsts. Pad slots use an out-of-bounds
   index (descriptor skipped by HW) and norm=0.
 - Device: per slot column, one indirect DMA gathers x[src] rows ([128,1] ->
   [128,64], the HW-supported form); DVE multiplies by norms and seg-reduces
   over slots; PE applies M = W @ softmax(CA) and the bias row.
"""
import os
import numpy as np

import concourse.bass as bass
import concourse.bacc as bacc
import concourse.mybir as mybir
import concourse.tile as tile
from concourse.bass_utils import run_bass_kernel_spmd

P = 128
D = 64
N_CORES = 8
OOB_IDX = 1 << 20

LAST_EXEC_NS = None


def _build_nc(N, n_blocks, s_list, col_off, ST):
    nc = bacc.Bacc(None, target_bir_lowering=False)
    f32 = mybir.dt.float32
    x = nc.declare_dram_parameter("x", [N, D], f32, isOutput=False)
    offs = nc.declare_dram_parameter("offs", [P, ST], mybir.dt.int32, isOutput=False)
    norms = nc.declare_dram_parameter("norms", [P, ST], f32, isOutput=False)
    wmat = nc.declare_dram_parameter("wmat", [D, D], f32, isOutput=False)
    bvec = nc.declare_dram_parameter("bvec", [D, 1], f32, isOutput=False)
    cattn = nc.declare_dram_parameter("cattn", [D, D], f32, isOutput=False)
    ident = nc.declare_dram_parameter("ident", [P, P], f32, isOutput=False)
    out = nc.declare_dram_parameter("out", [n_blocks * P, D], f32, isOutput=True)

    s_max = max(s_list)

    with tile.TileContext(nc) as tc:
        with (
            tc.tile_pool(name="const", bufs=1) as cpool,
            tc.tile_pool(name="psum", bufs=2, space="PSUM") as ppool,
            tc.tile_pool(name="work", bufs=3) as wpool,
            tc.tile_pool(name="outp", bufs=3) as opool,
        ):
            offs_s = cpool.tile([P, ST], mybir.dt.int32)
            norms_s = cpool.tile([P, ST], f32)
            nc.sync.dma_start(out=offs_s[:], in_=offs[:, :])
            nc.sync.dma_start(out=norms_s[:], in_=norms[:, :])
            id_s = cpool.tile([P, P], f32)
            nc.sync.dma_start(out=id_s[:], in_=ident[:, :])
            w_s = cpool.tile([D, D], f32)
            nc.sync.dma_start(out=w_s[:], in_=wmat[:, :])
            b_s = cpool.tile([D, 1], f32)
            nc.sync.dma_start(out=b_s[:], in_=bvec[:, :])
            ca_s = cpool.tile([D, D], f32)
            nc.sync.dma_start(out=ca_s[:], in_=cattn[:, :])

            # ---- softmax(CA, axis=1) in-place on ca_s ----
            mx = cpool.tile([D, 1], f32)
            nc.vector.tensor_reduce(out=mx[:], in_=ca_s[:], axis=mybir.AxisListType.X,
                                    op=mybir.AluOpType.max)
            nc.vector.tensor_scalar_mul(mx[:], mx[:], -1.0)
            nc.scalar.activation(out=ca_s[:], in_=ca_s[:],
                                 func=mybir.ActivationFunctionType.Exp,
                                 bias=mx[:, :1], scale=1.0)
            sm = cpool.tile([D, 1], f32)
            nc.vector.tensor_reduce(out=sm[:], in_=ca_s[:], axis=mybir.AxisListType.X,
                                    op=mybir.AluOpType.add)
            rc = cpool.tile([D, 1], f32)
            nc.vector.reciprocal(rc[:], sm[:])
            nc.vector.tensor_scalar_mul(ca_s[:], ca_s[:], rc[:, :1])

            # ---- M = W @ softmax(CA);  bS = b.T @ softmax(CA) ----
            wt_p = ppool.tile([D, D], f32, tag="pa")
            nc.tensor.transpose(wt_p[:], w_s[:], id_s[:D, :D])
            wt_s = cpool.tile([D, D], f32)
            nc.vector.tensor_copy(out=wt_s[:], in_=wt_p[:])
            m_p = ppool.tile([D, D], f32, tag="pa")
            nc.tensor.matmul(m_p[:], wt_s[:], ca_s[:], start=True, stop=True)
            m_s = cpool.tile([D, D], f32)
            nc.vector.tensor_copy(out=m_s[:], in_=m_p[:])
            bs_p = ppool.tile([1, D], f32, tag="pa")
            nc.tensor.matmul(bs_p[:], b_s[:, :1], ca_s[:], start=True, stop=True)
            bs_s = cpool.tile([1, D], f32)
            nc.vector.tensor_copy(out=bs_s[:], in_=bs_p[:])
            ones_s = cpool.tile([1, P], f32)
            nc.vector.memset(ones_s[:], 1.0)

            # ---- main loop over dst blocks ----
            for b in range(n_blocks):
                S = s_list[b]
                c0 = col_off[b]
                feat = wpool.tile([P, s_max * D], f32, tag="feat")
                if b < 3:
                    nc.vector.memset(feat[:], 0.0)
                for j in range(S):
                    nc.gpsimd.indirect_dma_start(
                        out=feat[:, j * D:(j + 1) * D],
                        out_offset=None,
                        in_=x[:, :],
                        in_offset=bass.IndirectOffsetOnAxis(
                            ap=offs_s[:, c0 + j:c0 + j + 1], axis=0),
                        bounds_check=N - 1,
                        oob_is_err=False,
                    )
                feat3 = feat[:, :S * D].rearrange("p (s d) -> p s d", s=S)
                nb = norms_s[:, c0:c0 + S].unsqueeze(2).to_broadcast([P, S, D])
                nc.vector.tensor_tensor(out=feat3, in0=feat3, in1=nb,
                                        op=mybir.AluOpType.mult)
                agg = opool.tile([P, D], f32, tag="agg")
                nc.vector.tensor_reduce(
                    out=agg[:], in_=feat[:, :S * D].rearrange("p (s d) -> p d s", s=S),
                    axis=mybir.AxisListType.X, op=mybir.AluOpType.add)
                # out_block = agg @ M + 1s*bS  (via aggT)
                t_p = ppool.tile([D, P], f32, tag="pt")
                nc.tensor.transpose(t_p[:], agg[:], id_s[:, :])
                aggT = opool.tile([D, P], f32, tag="aggT")
                nc.vector.tensor_copy(out=aggT[:], in_=t_p[:])
                o_p = ppool.tile([P, D], f32, tag="po")
                nc.tensor.matmul(o_p[:], aggT[:], m_s[:], start=True, stop=False)
                nc.tensor.matmul(o_p[:], ones_s[:, :], bs_s[:, :], start=False,
                                 stop=True, skip_group_check=True)
                o_s = opool.tile([P, D], f32, tag="os")
                nc.vector.tensor_copy(out=o_s[:], in_=o_p[:])
                nc.sync.dma_start(out=out[b * P:(b + 1) * P, :], in_=o_s[:])
    nc.compile()
    return nc


def kernel(x, edge_index, W, b, causal_attention, L=1, **_unused):
    global LAST_EXEC_NS
    x = np.ascontiguousarray(np.asarray(x, dtype=np.float32))
    ei = np.asarray(edge_index, dtype=np.int64)
    W = np.asarray(W, dtype=np.float32)
    bb = np.asarray(b, dtype=np.float32).reshape(D, 1)
    ca = np.asarray(causal_attention, dtype=np.float32)
    N = x.shape[0]
    src, dst = ei[0].astype(np.int64), ei[1].astype(np.int64)

    # GCN normalization (index-only math)
    deg = np.bincount(dst, minlength=N).astype(np.float64) + 1.0
    dinv = (1.0 / np.sqrt(deg)).astype(np.float32)
    norm_e = dinv[src] * dinv[dst]

    n_per = N // N_CORES
    n_blocks = (n_per + P - 1) // P

    # per-core degree-sorted dst ordering and slot tables
    cores = []
    for c in range(N_CORES):
        lo, hi = c * n_per, (c + 1) * n_per
        sel = (dst >= lo) & (dst < hi)
        s_c, d_c, w_c = src[sel], dst[sel] - lo, norm_e[sel]
        degc = np.bincount(d_c, minlength=n_per) + 1  # incl self loop
        order = np.argsort(-degc, kind="stable")      # dst local ids, degree desc
        rank = np.empty(n_per, np.int64)
        rank[order] = np.arange(n_per)
        cores.append((lo, s_c, d_c, w_c, degc, order, rank))

    # uniform per-block slot counts across cores
    s_list = []
    for bidx in range(n_blocks):
        m = 1
        for (_, _, _, _, degc, order, _) in cores:
            i0 = bidx * P
            if i0 < n_per:
                m = max(m, int(degc[order[i0]]))
        s_list.append(m)
    col_off = np.concatenate([[0], np.cumsum(s_list)]).astype(np.int64)
    ST = int(col_off[-1])

    in_maps = []
    perms = []
    for c in range(N_CORES):
        lo, s_c, d_c, w_c, degc, order, rank = cores[c]
        offs_arr = np.full((P, ST), OOB_IDX, dtype=np.int32)
        norms_arr = np.zeros((P, ST), dtype=np.float32)

        # self loops: slot 0 of every dst
        r_all = rank  # rank of local dst i
        p_all = (r_all % P).astype(np.int64)
        blk_all = r_all // P
        cols0 = col_off[blk_all]
        offs_arr[p_all, cols0] = (np.arange(n_per) + lo).astype(np.int32)
        norms_arr[p_all, cols0] = dinv[lo:lo + n_per] ** 2

        # edges: slots 1.. per dst in rank order
        rk = rank[d_c]
        o2 = np.argsort(rk, kind="stable")
        rk_s, s_s, w_s_ = rk[o2], s_c[o2], w_c[o2]
        # position within group
        grp_start = np.searchsorted(rk_s, np.arange(n_per), side="left")
        j_in = np.arange(len(rk_s)) - grp_start[rk_s]
        cols = col_off[rk_s // P] + 1 + j_in
        rows = rk_s % P
        offs_arr[rows, cols] = s_s.astype(np.int32)
        norms_arr[rows, cols] = w_s_

        in_maps.append({
            "x": x, "offs": offs_arr, "norms": norms_arr,
            "wmat": W, "bvec": bb, "cattn": ca,
            "ident": np.eye(P, dtype=np.float32),
        })
        perms.append(order + lo)

    nc = _build_nc(N, n_blocks, s_list, col_off, ST)

    trace = bool(os.environ.get("KERNEL_TRACE"))
    if trace:
        try:
            import ntff_shim  # noqa: F401
        except Exception:
            trace = False
    r = run_bass_kernel_spmd(nc, in_maps, list(range(N_CORES)), trace=trace)
    LAST_EXEC_NS = r.exec_time_ns

    out = np.empty((N, D), dtype=np.float32)
    for c in range(N_CORES):
        out[perms[c]] = r.results[c]["out"][:n_per]
    return out


# revision 2
# speedup vs baseline: 1.0456x; 1.0456x over previous
"""Trainium2 Bass kernel for CausalGraphLayer (GCN conv + causal attention mix).

out = D^{-1/2} (A+I) D^{-1/2} x @ (W @ softmax(CA, axis=1)) + b @ softmax(CA)

Strategy (8 NeuronCores, SPMD):
 - Shard destination nodes across cores (12500 each); partition edges by dst.
 - Replicate x and the small params to every core.
 - Host builds, per core, a slot table: dst nodes degree-sorted into blocks of
   128 (PSUM partitions); slot j of block b holds the j-th in-edge's source
   index and norm for each of the 128 dsts. Pad slots use an out-of-bounds
   index (descriptor skipped by HW) and norm=0.
 - Device: per slot column, one indirect DMA gathers x[src] rows ([128,1] ->
   [128,64], the HW-supported form); DVE multiplies by norms and seg-reduces
   over slots; PE applies M = W @ softmax(CA) and the bias row.
"""
import os
import numpy as np

NO_BC = bool(os.environ.get("KERNEL_NB"))

import concourse.bass as bass
import concourse.bacc as bacc
import concourse.mybir as mybir
import concourse.tile as tile
from concourse.bass_utils import run_bass_kernel_spmd

P = 128
D = 64
N_CORES = 8
OOB_IDX = 1 << 20

LAST_EXEC_NS = None


def _build_nc(N, n_blocks, s_list, col_off, ST):
    nc = bacc.Bacc(None, target_bir_lowering=False)
    f32 = mybir.dt.float32
    x = nc.declare_dram_parameter("x", [N, D], f32, isOutput=False)
    offs = nc.declare_dram_parameter("offs", [P, ST], mybir.dt.int32, isOutput=False)
    norms = nc.declare_dram_parameter("norms", [P, ST], f32, isOutput=False)
    wmat = nc.declare_dram_parameter("wmat", [D, D], f32, isOutput=False)
    bvec = nc.declare_dram_parameter("bvec", [D, 1], f32, isOutput=False)
    cattn = nc.declare_dram_parameter("cattn", [D, D], f32, isOutput=False)
    ident = nc.declare_dram_parameter("ident", [P, P], f32, isOutput=False)
    out = nc.declare_dram_parameter("out", [n_blocks * P, D], f32, isOutput=True)

    s_max = max(s_list)

    with tile.TileContext(nc) as tc:
        with (
            tc.tile_pool(name="const", bufs=1) as cpool,
            tc.tile_pool(name="psum", bufs=2, space="PSUM") as ppool,
            tc.tile_pool(name="work", bufs=3) as wpool,
            tc.tile_pool(name="outp", bufs=3) as opool,
        ):
            offs_s = cpool.tile([P, ST], mybir.dt.int32)
            norms_s = cpool.tile([P, ST], f32)
            nc.sync.dma_start(out=offs_s[:], in_=offs[:, :])
            nc.sync.dma_start(out=norms_s[:], in_=norms[:, :])
            id_s = cpool.tile([P, P], f32)
            nc.sync.dma_start(out=id_s[:], in_=ident[:, :])
            w_s = cpool.tile([D, D], f32)
            nc.sync.dma_start(out=w_s[:], in_=wmat[:, :])
            b_s = cpool.tile([D, 1], f32)
            nc.sync.dma_start(out=b_s[:], in_=bvec[:, :])
            ca_s = cpool.tile([D, D], f32)
            nc.sync.dma_start(out=ca_s[:], in_=cattn[:, :])

            # ---- softmax(CA, axis=1) in-place on ca_s ----
            mx = cpool.tile([D, 1], f32)
            nc.vector.tensor_reduce(out=mx[:], in_=ca_s[:], axis=mybir.AxisListType.X,
                                    op=mybir.AluOpType.max)
            nc.vector.tensor_scalar_mul(mx[:], mx[:], -1.0)
            nc.scalar.activation(out=ca_s[:], in_=ca_s[:],
                                 func=mybir.ActivationFunctionType.Exp,
                                 bias=mx[:, :1], scale=1.0)
            sm = cpool.tile([D, 1], f32)
            nc.vector.tensor_reduce(out=sm[:], in_=ca_s[:], axis=mybir.AxisListType.X,
                                    op=mybir.AluOpType.add)
            rc = cpool.tile([D, 1], f32)
            nc.vector.reciprocal(rc[:], sm[:])
            nc.vector.tensor_scalar_mul(ca_s[:], ca_s[:], rc[:, :1])

            # ---- M = W @ softmax(CA);  bS = b.T @ softmax(CA) ----
            wt_p = ppool.tile([D, D], f32, tag="pa")
            nc.tensor.transpose(wt_p[:], w_s[:], id_s[:D, :D])
            wt_s = cpool.tile([D, D], f32)
            nc.vector.tensor_copy(out=wt_s[:], in_=wt_p[:])
            m_p = ppool.tile([D, D], f32, tag="pa")
            nc.tensor.matmul(m_p[:], wt_s[:], ca_s[:], start=True, stop=True)
            m_s = cpool.tile([D, D], f32)
            nc.vector.tensor_copy(out=m_s[:], in_=m_p[:])
            bs_p = ppool.tile([1, D], f32, tag="pa")
            nc.tensor.matmul(bs_p[:], b_s[:, :1], ca_s[:], start=True, stop=True)
            bs_s = cpool.tile([1, D], f32)
            nc.vector.tensor_copy(out=bs_s[:], in_=bs_p[:])
            ones_s = cpool.tile([1, P], f32)
            nc.vector.memset(ones_s[:], 1.0)

            # ---- main loop over dst blocks ----
            for b in range(n_blocks):
                S = s_list[b]
                c0 = col_off[b]
                feat = wpool.tile([P, s_max * D], f32, tag="feat")
                if b < 3:
                    nc.vector.memset(feat[:], 0.0)
                for j in range(S):
                    nc.gpsimd.indirect_dma_start(
                        out=feat[:, j * D:(j + 1) * D],
                        out_offset=None,
                        in_=x[:, :],
                        in_offset=bass.IndirectOffsetOnAxis(
                            ap=offs_s[:, c0 + j:c0 + j + 1], axis=0),
                        bounds_check=None if NO_BC else N - 1,
                        oob_is_err=False,
                    )
                feat3 = feat[:, :S * D].rearrange("p (s d) -> p s d", s=S)
                nb = norms_s[:, c0:c0 + S].unsqueeze(2).to_broadcast([P, S, D])
                nc.vector.tensor_tensor(out=feat3, in0=feat3, in1=nb,
                                        op=mybir.AluOpType.mult)
                agg = opool.tile([P, D], f32, tag="agg")
                nc.vector.tensor_reduce(
                    out=agg[:], in_=feat[:, :S * D].rearrange("p (s d) -> p d s", s=S),
                    axis=mybir.AxisListType.X, op=mybir.AluOpType.add)
                # out_block = agg @ M + 1s*bS  (via aggT)
                t_p = ppool.tile([D, P], f32, tag="pt")
                nc.tensor.transpose(t_p[:], agg[:], id_s[:, :])
                aggT = opool.tile([D, P], f32, tag="aggT")
                nc.vector.tensor_copy(out=aggT[:], in_=t_p[:])
                o_p = ppool.tile([P, D], f32, tag="po")
                nc.tensor.matmul(o_p[:], aggT[:], m_s[:], start=True, stop=False)
                nc.tensor.matmul(o_p[:], ones_s[:, :], bs_s[:, :], start=False,
                                 stop=True, skip_group_check=True)
                o_s = opool.tile([P, D], f32, tag="os")
                nc.vector.tensor_copy(out=o_s[:], in_=o_p[:])
                nc.sync.dma_start(out=out[b * P:(b + 1) * P, :], in_=o_s[:])
    nc.compile()
    return nc


def kernel(x, edge_index, W, b, causal_attention, L=1, **_unused):
    global LAST_EXEC_NS
    x = np.ascontiguousarray(np.asarray(x, dtype=np.float32))
    ei = np.asarray(edge_index, dtype=np.int64)
    W = np.asarray(W, dtype=np.float32)
    bb = np.asarray(b, dtype=np.float32).reshape(D, 1)
    ca = np.asarray(causal_attention, dtype=np.float32)
    N = x.shape[0]
    src, dst = ei[0].astype(np.int64), ei[1].astype(np.int64)

    # GCN normalization (index-only math)
    deg = np.bincount(dst, minlength=N).astype(np.float64) + 1.0
    dinv = (1.0 / np.sqrt(deg)).astype(np.float32)
    norm_e = dinv[src] * dinv[dst]

    n_per = N // N_CORES
    n_blocks = (n_per + P - 1) // P

    # per-core degree-sorted dst ordering and slot tables
    cores = []
    for c in range(N_CORES):
        lo, hi = c * n_per, (c + 1) * n_per
        sel = (dst >= lo) & (dst < hi)
        s_c, d_c, w_c = src[sel], dst[sel] - lo, norm_e[sel]
        degc = np.bincount(d_c, minlength=n_per) + 1  # incl self loop
        order = np.argsort(-degc, kind="stable")      # dst local ids, degree desc
        rank = np.empty(n_per, np.int64)
        rank[order] = np.arange(n_per)
        cores.append((lo, s_c, d_c, w_c, degc, order, rank))

    # uniform per-block slot counts across cores
    s_list = []
    for bidx in range(n_blocks):
        m = 1
        for (_, _, _, _, degc, order, _) in cores:
            i0 = bidx * P
            if i0 < n_per:
                m = max(m, int(degc[order[i0]]))
        s_list.append(m)
    col_off = np.concatenate([[0], np.cumsum(s_list)]).astype(np.int64)
    ST = int(col_off[-1])

    in_maps = []
    perms = []
    for c in range(N_CORES):
        lo, s_c, d_c, w_c, degc, order, rank = cores[c]
        offs_arr = np.full((P, ST), 0 if NO_BC else OOB_IDX, dtype=np.int32)
        norms_arr = np.zeros((P, ST), dtype=np.float32)

        # self loops: slot 0 of every dst
        r_all = rank  # rank of local dst i
        p_all = (r_all % P).astype(np.int64)
        blk_all = r_all // P
        cols0 = col_off[blk_all]
        offs_arr[p_all, cols0] = (np.arange(n_per) + lo).astype(np.int32)
        norms_arr[p_all, cols0] = dinv[lo:lo + n_per] ** 2

        # edges: slots 1.. per dst in rank order
        rk = rank[d_c]
        o2 = np.argsort(rk, kind="stable")
        rk_s, s_s, w_s_ = rk[o2], s_c[o2], w_c[o2]
        # position within group
        grp_start = np.searchsorted(rk_s, np.arange(n_per), side="left")
        j_in = np.arange(len(rk_s)) - grp_start[rk_s]
        cols = col_off[rk_s // P] + 1 + j_in
        rows = rk_s % P
        offs_arr[rows, cols] = s_s.astype(np.int32)
        norms_arr[rows, cols] = w_s_

        in_maps.append({
            "x": x, "offs": offs_arr, "norms": norms_arr,
            "wmat": W, "bvec": bb, "cattn": ca,
            "ident": np.eye(P, dtype=np.float32),
        })
        perms.append(order + lo)

    nc = _build_nc(N, n_blocks, s_list, col_off, ST)

    trace = bool(os.environ.get("KERNEL_TRACE"))
    if trace:
        try:
            import ntff_shim  # noqa: F401
        except Exception:
            trace = False
    r = run_bass_kernel_spmd(nc, in_maps, list(range(N_CORES)), trace=trace)
    LAST_EXEC_NS = r.exec_time_ns

    out = np.empty((N, D), dtype=np.float32)
    for c in range(N_CORES):
        out[perms[c]] = r.results[c]["out"][:n_per]
    return out
